# revision 8
# baseline (speedup 1.0000x reference)
"""Trainium2 kernel for nn_CCQC_classifier.

The reference applies a fixed 10-qubit/depth-5 circuit U (built only from the
tiny weight tensors) to each normalized, zero-padded input row, then reads out
logits l_k = <x|U^H Z_k U|x> / |x|^2 for k in {0,1} and returns mean NLL.

Since log_softmax over 2 classes depends only on the logit difference,
    nll_b = softplus((2*y_b - 1) * x_b^T M_d x_b / |x_b|^2)
with M_d = Re(U^H diag(z0 - z1) U)[:784, :784] a single fixed 784x784 real
symmetric matrix. The host builds M_d from the weights (cheap, data
independent); the device computes, per 1024-row batch shard:
    Y = X @ M_d   (bf16 matmul, fp32 accum)
    e = rowsum(Y * X),  n2 = rowsum(X * X)
    nll = softplus(sgn * e / n2),  out = sum(nll)
Data-parallel across 8 NeuronCores; host sums the 8 partials / 8192.
"""

import sys

import numpy as np

for _p in ("/opt/trn_rl_repo", "/root/.axon_site/_ro/trn_rl_repo"):
    if _p not in sys.path:
        sys.path.append(_p)

N_QUBITS = 10
DEPTH = 5
DIM = 2**N_QUBITS  # 1024
F = 784  # true feature dim (rest of the 1024 state is zero padded)
B = 8192
NCORES = 8
BC = B // NCORES  # 1024 rows per core
P = 128
KT = 7  # contraction tiles: 784 padded to 896 = 7*128
NB = BC // P  # 8 batch chunks per core
FPAD = KT * P  # 896


# ---------------------------------------------------------------- host math
def _apply_1q(state, U, w):
    bdim = state.shape[0]
    s = state.reshape(bdim, 2**w, 2, 2 ** (N_QUBITS - 1 - w))
    s0 = s[:, :, 0, :].copy()
    s1 = s[:, :, 1, :].copy()
    s[:, :, 0, :] = U[0, 0] * s0 + U[0, 1] * s1
    s[:, :, 1, :] = U[1, 0] * s0 + U[1, 1] * s1
    return state


def _apply_c1q(state, U, ctrl, tgt):
    idx = np.arange(DIM)
    cbit = (idx >> (N_QUBITS - 1 - ctrl)) & 1
    tbit = (idx >> (N_QUBITS - 1 - tgt)) & 1
    tstride = 1 << (N_QUBITS - 1 - tgt)
    i0 = idx[(cbit == 1) & (tbit == 0)]
    i1 = i0 + tstride
    s0 = state[:, i0].copy()
    s1 = state[:, i1]
    state[:, i0] = U[0, 0] * s0 + U[0, 1] * s1
    state[:, i1] = U[1, 0] * s0 + U[1, 1] * s1
    return state


def _rx(t):
    c, s = np.cos(t / 2), np.sin(t / 2)
    return np.array([[c, -1j * s], [-1j * s, c]])


def _rz(t):
    e = np.exp(-1j * t / 2)
    return np.array([[e, 0], [0, np.conj(e)]])


def _build_Md(weights, weights_1, weights_2):
    """M_d = Re(U^H diag(z0-z1) U)[:784,:784] for the CCQC circuit."""
    weights = np.asarray(weights, np.float64)
    weights_1 = np.asarray(weights_1, np.float64)
    weights_2 = np.asarray(weights_2, np.float64)
    # state[b, :] = U @ e_b, so state = U^T as a matrix
    state = np.eye(DIM, dtype=np.complex128)
    for d in range(DEPTH):
        for i in range(N_QUBITS):
            state = _apply_1q(state, _rx(weights[d, i, 0]), i)
            state = _apply_1q(state, _rz(weights[d, i, 1]), i)
            state = _apply_1q(state, _rx(weights[d, i, 2]), i)
        r = 1 if d % 2 == 0 else 3
        for i in range(N_QUBITS):
            c = (i + r) % N_QUBITS
            state = _apply_c1q(state, _rz(weights[d, i, 3]), c, i)
            state = _apply_c1q(state, _rx(weights[d, i, 4]), c, i)
        state = _apply_1q(state, _rx(weights_1[d]), 0)
        state = _apply_1q(state, _rz(weights_2[d]), 0)
    # U[j, b] = state[b, j]
    idx = np.arange(DIM)
    zd = (2 * ((idx >> 8) & 1) - 2 * ((idx >> 9) & 1)).astype(np.float64)
    mask = zd != 0
    zsel = zd[mask]
    # columns of U (in-dim) live in state's row index b; restrict b < 784
    Ur = np.ascontiguousarray(state.real[:F, mask])  # (784, 512) = U.real[mask,:784].T
    Ui = np.ascontiguousarray(state.imag[:F, mask])
    Md = Ur @ (zsel[:, None] * Ur.T) + Ui @ (zsel[:, None] * Ui.T)
    return Md  # (784, 784) float64 symmetric


# ---------------------------------------------------------------- device code
_CACHE = {}


def _build_bass():
    import concourse.bacc as bacc
    import concourse.tile as tile
    from concourse import mybir

    f32 = mybir.dt.float32
    bf16 = mybir.dt.bfloat16

    nc = bacc.Bacc()
    xt_d = nc.dram_tensor("xt", (P, KT, BC), bf16, kind="ExternalInput")
    mb_d = nc.dram_tensor("mb", (P, KT, F), bf16, kind="ExternalInput")
    xb_d = nc.dram_tensor("xb", (P, NB, F), bf16, kind="ExternalInput")
    sgn_d = nc.dram_tensor("sgn", (P, NB), f32, kind="ExternalInput")
    out_d = nc.dram_tensor("out", (1, 1), f32, kind="ExternalOutput")

    NSPLITS = ((0, 512), (512, F))

    with tile.TileContext(nc) as tc:
        with (
            tc.tile_pool(name="const", bufs=1) as cpool,
            tc.tile_pool(name="scratch", bufs=3) as spool,
            tc.tile_pool(name="psum", bufs=3, space="PSUM") as psum,
            tc.tile_pool(name="psum_s", bufs=1, space="PSUM") as psum_s,
        ):
            xt = cpool.tile([P, KT, BC], bf16)
            mb = cpool.tile([P, KT, F], bf16)
            xb = cpool.tile([P, NB, F], bf16)
            sgn = cpool.tile([P, NB], f32)
            nc.sync.dma_start(out=mb[:], in_=mb_d[:])
            nc.sync.dma_start(out=xt[:], in_=xt_d[:])
            nc.sync.dma_start(out=xb[:], in_=xb_d[:])
            nc.sync.dma_start(out=sgn[:], in_=sgn_d[:])

            ones = cpool.tile([P, 1], f32)
            nc.vector.memset(ones[:], 1.0)

            e = cpool.tile([P, NB], f32)
            n2 = cpool.tile([P, NB], f32)

            for i in range(NB):
                y_ps = psum.tile([P, F], f32)
                for n0, n1 in NSPLITS:
                    for k in range(KT):
                        nc.tensor.matmul(
                            y_ps[:, n0:n1],
                            lhsT=xt[:, k, i * P : (i + 1) * P],
                            rhs=mb[:, k, n0:n1],
                            start=(k == 0),
                            stop=(k == KT - 1),
                        )
                # n2[:, i] = rowsum(x^2) on ScalarE (fused square+accum)
                scr_a = spool.tile([P, F], bf16, tag="scr_a")
                nc.scalar.activation(
                    out=scr_a[:],
                    in_=xb[:, i, :],
                    func=mybir.ActivationFunctionType.Square,
                    accum_out=n2[:, i : i + 1],
                )
                # e[:, i] = rowsum(Y * x): product on VectorE, reduce on ScalarE
                prod = spool.tile([P, F], f32, tag="prod")
                nc.vector.tensor_mul(prod[:], y_ps[:], xb[:, i, :])
                scr_e = spool.tile([P, F], bf16, tag="scr_e")
                nc.scalar.activation(
                    out=scr_e[:],
                    in_=prod[:],
                    func=mybir.ActivationFunctionType.Identity,
                    accum_out=e[:, i : i + 1],
                )

            # arg = sgn * e / n2 ; nll = softplus(arg) ; partial = sum(nll)
            rn2 = cpool.tile([P, NB], f32)
            nc.vector.reciprocal(out=rn2[:], in_=n2[:])
            arg = cpool.tile([P, NB], f32)
            nc.vector.tensor_mul(arg[:], e[:], rn2[:])
            nc.vector.tensor_mul(arg[:], arg[:], sgn[:])
            # softplus(arg) = ln(1 + exp(arg)); Softplus itself fails lower_act
            expv = cpool.tile([P, NB], f32)
            nc.scalar.activation(
                out=expv[:], in_=arg[:], func=mybir.ActivationFunctionType.Exp
            )
            nll = cpool.tile([P, NB], f32)
            nllsum = cpool.tile([P, 1], f32)
            nc.scalar.activation(
                out=nll[:],
                in_=expv[:],
                func=mybir.ActivationFunctionType.Ln,
                bias=1.0,
                accum_out=nllsum[:],
            )
            # cross-partition sum via matmul with ones
            tot_ps = psum_s.tile([1, 1], f32)
            nc.tensor.matmul(tot_ps[:], lhsT=nllsum[:], rhs=ones[:], start=True, stop=True)
            res = cpool.tile([1, 1], f32)
            nc.vector.tensor_copy(res[:], tot_ps[:])
            nc.sync.dma_start(out=out_d[:], in_=res[:])

    nc.finalize()
    return nc


def kernel(x, y, weights, weights_1, weights_2):
    import ml_dtypes

    from concourse.bass_utils import run_bass_kernel_spmd

    x = np.asarray(x, np.float32)
    y = np.asarray(y)

    Md = _build_Md(weights, weights_1, weights_2)

    if "nc" not in _CACHE:
        _CACHE["nc"] = _build_bass()
    nc = _CACHE["nc"]

    bf16 = ml_dtypes.bfloat16
    # M_d padded to (896, 784) -> (P, KT, F): mb[p, k, :] = Md[k*128+p, :]
    Mpad = np.zeros((FPAD, F), np.float32)
    Mpad[:F] = Md.astype(np.float32)
    mb_host = np.ascontiguousarray(
        Mpad.reshape(KT, P, F).transpose(1, 0, 2).astype(bf16)
    )

    sgn_full = (2.0 * np.asarray(y, np.float64) - 1.0).astype(np.float32)

    in_maps = []
    for c in range(NCORES):
        xs = x[c * BC : (c + 1) * BC]  # (1024, 784)
        xsb = xs.astype(bf16)
        # xt[p, k, b] = x[b, k*128+p] ; zero-pad features 784..895
        xtt = np.ascontiguousarray(xsb.T)  # (784, 1024)
        xt3 = np.zeros((KT * P, BC), bf16)
        xt3[:F] = xtt
        xt_host = np.ascontiguousarray(xt3.reshape(KT, P, BC).transpose(1, 0, 2))
        # xb[p, i, :] = x[i*128+p, :]
        xb_host = np.ascontiguousarray(xsb.reshape(NB, P, F).transpose(1, 0, 2))
        # sgn[p, i] = 2*y[i*128+p]-1
        sg = sgn_full[c * BC : (c + 1) * BC]
        sgn_host = np.ascontiguousarray(sg.reshape(NB, P).T)
        in_maps.append(
            {"xt": xt_host, "mb": mb_host, "xb": xb_host, "sgn": sgn_host}
        )

    res = run_bass_kernel_spmd(nc, in_maps, core_ids=list(range(NCORES)))
    _CACHE["last"] = res  # test harness reads exec_time_ns/profile from here
    total = sum(float(r["out"][0, 0]) for r in res.results)
    return np.array(total / B, dtype=np.float32)
